# revision 1
# baseline (speedup 1.0000x reference)
"""Trainium2 kernel for nn_AssocScan: out[t] = gates[t]*out[t-1] + inputs[t].

Full shapes: gates/inputs/out = (4, 8192, 1024) float32.

Strategy: the scan is independent per (b, d) lane; only the sequence
dim carries the recurrence. Shard d 8-ways across the NeuronCores
(128 d-lanes per core = exactly the 128 SBUF partitions), keep all of
b and the sequence on each core. Host-side, transpose to (d, b*n) so
each core's shard is a contiguous [128, 32768] block whose partition
rows are DMA-friendly contiguous sequences.

On-core: the recurrence runs along the free dim via the DVE
tensor_tensor_scan instruction (op0=mult, op1=add) at line rate,
tiled along the sequence with chained initial state (initial = last
column of the previous output tile). The scan runs in-place over the
x tile. Loads stream on the SP HWDGE ring, stores on the ACT ring.
No cross-core communication is needed.
"""

import numpy as np

B, N, D = 4, 8192, 1024
NCORES = 8
P = D // NCORES        # 128 partitions per core
T = 4096               # sequence tile (free-dim) size
BUFS = 4

_NC = None


def _build():
    import concourse.bacc as bacc
    import concourse.mybir as mybir
    from concourse.tile import TileContext

    f32 = mybir.dt.float32
    nc = bacc.Bacc()
    g = nc.declare_dram_parameter("gates", [P, B * N], f32, isOutput=False)
    x = nc.declare_dram_parameter("inputs", [P, B * N], f32, isOutput=False)
    o = nc.declare_dram_parameter("out", [P, B * N], f32, isOutput=True)
    NT = N // T
    with TileContext(nc) as tc:
        with tc.tile_pool(name="pool", bufs=BUFS) as pool:
            for b in range(B):
                prev = None
                for k in range(NT):
                    off = b * N + k * T
                    gt = pool.tile([P, T], f32, tag="g")
                    xt = pool.tile([P, T], f32, tag="x")
                    nc.sync.dma_start(out=gt[:, :], in_=g[:, off:off + T])
                    nc.sync.dma_start(out=xt[:, :], in_=x[:, off:off + T])
                    init = 0.0 if k == 0 else prev[:, T - 1:T]
                    nc.vector.tensor_tensor_scan(
                        out=xt[:, :],
                        data0=gt[:, :],
                        data1=xt[:, :],
                        initial=init,
                        op0=mybir.AluOpType.mult,
                        op1=mybir.AluOpType.add,
                    )
                    nc.scalar.dma_start(out=o[:, off:off + T], in_=xt[:, :])
                    prev = xt
    nc.compile()
    return nc


def get_nc():
    global _NC
    if _NC is None:
        _NC = _build()
    return _NC


def _shard(arr):
    # (B, N, D) -> (D, B*N) contiguous, then split into 8 row blocks
    t = np.ascontiguousarray(arr.reshape(B * N, D).T)
    return [t[i * P:(i + 1) * P] for i in range(NCORES)]


def kernel(gates, inputs):
    from concourse.bass_utils import run_bass_kernel_spmd

    gates = np.asarray(gates, dtype=np.float32)
    inputs = np.asarray(inputs, dtype=np.float32)
    g_shards = _shard(gates)
    x_shards = _shard(inputs)
    in_maps = [
        {"gates": g_shards[i], "inputs": x_shards[i]} for i in range(NCORES)
    ]
    res = run_bass_kernel_spmd(get_nc(), in_maps, core_ids=list(range(NCORES)))
    out_t = np.concatenate([res.results[i]["out"] for i in range(NCORES)], axis=0)
    return np.ascontiguousarray(out_t.T).reshape(B, N, D)
